# revision 19
# baseline (speedup 1.0000x reference)
"""Trainium2 Bass kernel for nn_Encoder (6-layer causal transformer encoder).

Sharding: 8 cores = 4 batch elements x 2-core tensor-parallel pairs.
Within a pair: attention is head-split (4 of 8 heads per core), FFN/LN/residual
are token-split (1024 of 2048 tokens per core).  Rank asymmetry is expressed
purely through ReduceScatter / AllGather rank order, so the SPMD program is
identical on every core.

Layout: activations are kept feature-major ("xT": [D on partitions, T free]),
which lets every matmul consume its operands without PE transposes and keeps
softmax score blocks in the AV-friendly [k, q] orientation.
"""

import os
import sys

sys.path.insert(0, "/opt/trn_rl_repo")

import numpy as np
import ml_dtypes

import concourse.bass as bass
import concourse.mybir as mybir
import concourse.tile as tile
from concourse import bacc, bass_utils
from concourse.masks import make_identity, make_upper_triangular

# Problem constants (hardcoded per harness contract).
B, S, V, D, F, L = 4, 2048, 32000, 512, 2048, 6
H, Dh = 8, 64
HL = H // 2            # local heads per core (4)
DL = HL * Dh           # 256 local head-dims
TOWN = S // 2          # 1024 tokens owned per core
P = 128
CC = D // P            # 4 c-chunks
FC = F // P            # 16 f-chunks
LN_EPS = 1e-5

FP32 = mybir.dt.float32
BF16 = mybir.dt.bfloat16
I32 = mybir.dt.int32

GROUPS = [[0, 1], [2, 3], [4, 5], [6, 7]]

_CACHED = {}


def _build_program():
    nc = bacc.Bacc("TRN2", target_bir_lowering=False, debug=False, num_devices=8)

    D_ = {}
    D_["src"] = nc.dram_tensor("src", [TOWN], I32, kind="ExternalInput")
    D_["emb"] = nc.dram_tensor("emb", [V, D], FP32, kind="ExternalInput")
    D_["wq"] = nc.dram_tensor("wq", [L, D, DL], BF16, kind="ExternalInput")
    D_["wk"] = nc.dram_tensor("wk", [L, D, DL], BF16, kind="ExternalInput")
    D_["wv"] = nc.dram_tensor("wv", [L, D, DL], BF16, kind="ExternalInput")
    D_["wo"] = nc.dram_tensor("wo", [L, DL, D], BF16, kind="ExternalInput")
    D_["bq"] = nc.dram_tensor("bq", [L, DL], FP32, kind="ExternalInput")
    D_["bk"] = nc.dram_tensor("bk", [L, DL], FP32, kind="ExternalInput")
    D_["bv"] = nc.dram_tensor("bv", [L, DL], FP32, kind="ExternalInput")
    D_["bo"] = nc.dram_tensor("bo", [L, D], FP32, kind="ExternalInput")
    D_["w1"] = nc.dram_tensor("w1", [L, D, F], BF16, kind="ExternalInput")
    D_["b1"] = nc.dram_tensor("b1", [L, F], FP32, kind="ExternalInput")
    D_["w2"] = nc.dram_tensor("w2", [L, F, D], BF16, kind="ExternalInput")
    D_["b2"] = nc.dram_tensor("b2", [L, D], FP32, kind="ExternalInput")
    D_["ln_g"] = nc.dram_tensor("ln_g", [D], FP32, kind="ExternalInput")
    D_["ln_b"] = nc.dram_tensor("ln_b", [D], FP32, kind="ExternalInput")
    D_["out"] = nc.dram_tensor("out", [TOWN, D], FP32, kind="ExternalOutput")

    # DRAM scratch (one set per layer so layers can overlap freely).
    # Collectives are split into two t-sub-halves (a/b) to overlap with compute.
    HT = TOWN // 2
    for s in ("a", "b"):
        D_[f"xh{s}"] = [nc.dram_tensor(f"xh{s}{l}", [D, HT], BF16, kind="Internal")
                        for l in range(L)]
        D_[f"xf{s}"] = [nc.dram_tensor(f"xf{s}{l}", [2, D, HT], BF16, kind="Internal")
                        for l in range(L)]
        D_[f"apart{s}"] = [nc.dram_tensor(f"apart{s}{l}", [2, D, HT], FP32, kind="Internal")
                           for l in range(L)]
        D_[f"aown{s}"] = [nc.dram_tensor(f"aown{s}{l}", [D, HT], FP32, kind="Internal")
                          for l in range(L)]
    D_["stb"] = [nc.dram_tensor(f"stb{l}", [2, TOWN], FP32, kind="Internal")
                 for l in range(2 * L)]
    D_["rcb"] = [nc.dram_tensor(f"rcb{l}", [HL, S], FP32, kind="Internal")
                 for l in range(L)]

    with tile.TileContext(nc) as tc:
        _emit(nc, tc, D_)

    nc.compile()
    return nc


def _emit(nc, tc, D_):
    from contextlib import ExitStack

    ctx = ExitStack()
    HT = TOWN // 2
    Exp = mybir.ActivationFunctionType.Exp
    Relu = mybir.ActivationFunctionType.Relu
    Sqrt = mybir.ActivationFunctionType.Sqrt
    ADD = mybir.AluOpType.add
    MULT = mybir.AluOpType.mult

    consts = ctx.enter_context(tc.tile_pool(name="consts", bufs=1))
    wpool = ctx.enter_context(tc.tile_pool(name="weights", bufs=1))
    wstrm = ctx.enter_context(tc.tile_pool(name="wstrm", bufs=3))
    stream = ctx.enter_context(tc.tile_pool(name="stream", bufs=1))
    acts = ctx.enter_context(tc.tile_pool(name="acts", bufs=1))
    halves = ctx.enter_context(tc.tile_pool(name="halves", bufs=1))
    small = ctx.enter_context(tc.tile_pool(name="small", bufs=2))
    expp = ctx.enter_context(tc.tile_pool(name="exp", bufs=4))
    bcast = ctx.enter_context(tc.tile_pool(name="bcast", bufs=2))
    psA = ctx.enter_context(tc.tile_pool(name="psA", bufs=2, space="PSUM"))
    psB = ctx.enter_context(tc.tile_pool(name="psB", bufs=2, space="PSUM"))
    psC = ctx.enter_context(tc.tile_pool(name="psC", bufs=2, space="PSUM"))

    # ---- constants ----
    ident = consts.tile([P, P], FP32)
    make_identity(nc, ident)
    trimask = consts.tile([P, P], BF16)  # 1 where k<=q
    make_upper_triangular(nc, trimask, val=1.0, diag=True)
    gT = consts.tile([P, CC], FP32)
    nc.sync.dma_start(out=gT, in_=D_["ln_g"].ap().rearrange("(cc p) -> p cc", p=P))
    bT = consts.tile([P, CC], FP32)
    nc.sync.dma_start(out=bT, in_=D_["ln_b"].ap().rearrange("(cc p) -> p cc", p=P))
    ones1 = consts.tile([P, 1], BF16)
    nc.vector.memset(ones1, 1.0)
    epst = consts.tile([1, 1], FP32)
    nc.vector.memset(epst, LN_EPS)
    idx = consts.tile([P, TOWN // P], I32)
    nc.sync.dma_start(out=idx, in_=D_["src"].ap().rearrange("(tc p) -> p tc", p=P))

    # ---- layer norm on own half, feature-major ----
    # s [P, CC, TOWN] fp32 -> out32 (fp32), outb (bf16 or None)
    def layer_norm(s, stb, out32, outb):
        for tg in range(TOWN // 512):
            sl = slice(tg * 512, (tg + 1) * 512)
            ps_m = psC.tile([1, 512], FP32, tag="psC", name="ps_m")
            ps_q = psC.tile([1, 512], FP32, tag="psC", name="ps_q")
            for cc in range(CC):
                chunk = small.tile([P, 512], BF16, tag="ln_chunk")
                nc.vector.tensor_copy(out=chunk, in_=s[:, cc, sl])
                sqc = small.tile([P, 512], BF16, tag="ln_sqc")
                nc.vector.tensor_mul(out=sqc, in0=chunk, in1=chunk)
                nc.tensor.matmul(ps_m, ones1, chunk, start=(cc == 0), stop=(cc == CC - 1))
                nc.tensor.matmul(ps_q, ones1, sqc, start=(cc == 0), stop=(cc == CC - 1))
            mean = small.tile([1, 512], FP32, tag="ln_mean", bufs=2)
            nc.scalar.mul(out=mean, in_=ps_m, mul=1.0 / D)
            msq = small.tile([1, 512], FP32, tag="ln_msq", bufs=1)
            nc.scalar.mul(out=msq, in_=ps_q, mul=1.0 / D)  # E[x^2]
            m2 = small.tile([1, 512], FP32, tag="ln_m2", bufs=1)
            nc.vector.tensor_mul(out=m2, in0=mean, in1=mean)
            nc.vector.tensor_sub(out=msq, in0=msq, in1=m2)
            rstd = small.tile([1, 512], FP32, tag="ln_rstd", bufs=2)
            nc.scalar.activation(out=rstd, in_=msq, func=Sqrt, bias=epst, scale=1.0)
            nc.vector.reciprocal(out=rstd, in_=rstd)
            nc.sync.dma_start(out=stb.ap()[0:1, sl], in_=mean)
            nc.sync.dma_start(out=stb.ap()[1:2, sl], in_=rstd)
        for tg in range(TOWN // 512):
            sl = slice(tg * 512, (tg + 1) * 512)
            mB = bcast.tile([P, 512], FP32, tag="mB")
            nc.sync.dma_start(out=mB, in_=bass.AP(tensor=stb, offset=tg * 512,
                                                  ap=[[0, P], [1, 512]]))
            rB = bcast.tile([P, 512], FP32, tag="rB")
            nc.sync.dma_start(out=rB, in_=bass.AP(tensor=stb, offset=TOWN + tg * 512,
                                                  ap=[[0, P], [1, 512]]))
            for cc in range(CC):
                o = out32[:, cc, sl]
                nc.vector.tensor_sub(out=o, in0=s[:, cc, sl], in1=mB)
                nc.vector.tensor_mul(out=o, in0=o, in1=rB)
                nc.vector.tensor_scalar(out=o, in0=o,
                                        scalar1=gT[:, cc:cc + 1], scalar2=bT[:, cc:cc + 1],
                                        op0=MULT, op1=ADD)
                if outb is not None:
                    nc.vector.tensor_copy(out=outb[:, cc, sl], in_=o)

    # ---- embedding gather for own tokens -> x_own [P, CC, TOWN] fp32 ----
    x_own = stream.tile([P, CC, TOWN], FP32, tag="x_own")
    xhb = halves.tile([P, CC, TOWN], BF16, tag="xhb")
    for tcN in range(TOWN // P):
        rows = acts.tile([P, D], FP32, tag="rows")
        nc.gpsimd.indirect_dma_start(
            out=rows, out_offset=None, in_=D_["emb"].ap(),
            in_offset=bass.IndirectOffsetOnAxis(ap=idx[:, tcN:tcN + 1], axis=0))
        for cc in range(CC):
            pt = psC.tile([P, P], FP32, tag="psC")
            nc.tensor.transpose(pt, rows[:, cc * P:(cc + 1) * P], ident)
            nc.vector.tensor_copy(out=x_own[:, cc, tcN * P:(tcN + 1) * P], in_=pt)
            nc.vector.tensor_copy(out=xhb[:, cc, tcN * P:(tcN + 1) * P], in_=pt)
    for s, sub in (("a", 0), ("b", 1)):
        nc.sync.dma_start(out=D_[f"xh{s}"][0].ap().rearrange("(cc p) t -> p cc t", p=P),
                          in_=xhb[:, :, sub * HT:(sub + 1) * HT])
        nc.gpsimd.collective_compute(
            kind="AllGather", op=mybir.AluOpType.bypass, replica_groups=GROUPS,
            ins=[D_[f"xh{s}"][0].ap()], outs=[D_[f"xf{s}"][0].ap()])

    for l in range(L):
        # ---- per-layer weights (small ones resident; w1/w2 streamed) ----
        wq_t = wpool.tile([P, CC, DL], BF16, tag="wq")
        nc.sync.dma_start(out=wq_t, in_=D_["wq"].ap()[l].rearrange("(cc p) d -> p cc d", p=P))
        wk_t = wpool.tile([P, CC, DL], BF16, tag="wk")
        nc.sync.dma_start(out=wk_t, in_=D_["wk"].ap()[l].rearrange("(cc p) d -> p cc d", p=P))
        wv_t = wpool.tile([P, CC, DL], BF16, tag="wv")
        nc.sync.dma_start(out=wv_t, in_=D_["wv"].ap()[l].rearrange("(cc p) d -> p cc d", p=P))
        wo_t = wpool.tile([P, 2, D], BF16, tag="wo")
        nc.sync.dma_start(out=wo_t, in_=D_["wo"].ap()[l].rearrange("(hc p) d -> p hc d", p=P))
        bq_t = wpool.tile([P, 2], FP32, tag="bq")
        nc.sync.dma_start(out=bq_t, in_=D_["bq"].ap()[l].rearrange("(hc p) -> p hc", p=P))
        bk_t = wpool.tile([P, 2], FP32, tag="bk")
        nc.sync.dma_start(out=bk_t, in_=D_["bk"].ap()[l].rearrange("(hc p) -> p hc", p=P))
        bvB = wpool.tile([P, DL], FP32, tag="bvB")
        nc.sync.dma_start(out=bvB, in_=bass.AP(tensor=D_["bv"], offset=l * DL,
                                               ap=[[0, P], [1, DL]]))
        bo_t = wpool.tile([P, CC], FP32, tag="bo")
        nc.sync.dma_start(out=bo_t, in_=D_["bo"].ap()[l].rearrange("(cc p) -> p cc", p=P))
        b1_t = wpool.tile([P, FC], FP32, tag="b1")
        nc.sync.dma_start(out=b1_t, in_=D_["b1"].ap()[l].rearrange("(fc p) -> p fc", p=P))
        b2_t = wpool.tile([P, CC], FP32, tag="b2")
        nc.sync.dma_start(out=b2_t, in_=D_["b2"].ap()[l].rearrange("(cc p) -> p cc", p=P))

        # ---- gathered x (full sequence, bf16) ----
        # global t order: [half0_sub_a, half0_sub_b, half1_sub_a, half1_sub_b]
        xb = acts.tile([P, CC, S], BF16, tag="xb")
        for half in range(2):
            for s, sub in (("a", 0), ("b", 1)):
                o = half * TOWN + sub * HT
                nc.sync.dma_start(
                    out=xb[:, :, o:o + HT],
                    in_=D_[f"xf{s}"][l].ap()[half].rearrange("(cc p) t -> p cc t", p=P))

        # ---- QKV projections ----
        QT = acts.tile([P, 2, S], BF16, tag="QT")
        KT = acts.tile([P, 2, S], BF16, tag="KT")
        for dst, w_t, b_t in ((QT, wq_t, bq_t), (KT, wk_t, bk_t)):
            for hc in range(2):
                for tg in range(S // 512):
                    ps = psC.tile([P, 512], FP32, tag="psC")
                    for cc in range(CC):
                        nc.tensor.matmul(
                            ps, w_t[:, cc, hc * P:(hc + 1) * P],
                            xb[:, cc, tg * 512:(tg + 1) * 512],
                            start=(cc == 0), stop=(cc == CC - 1))
                    nc.vector.tensor_scalar_add(
                        out=dst[:, hc, tg * 512:(tg + 1) * 512], in0=ps,
                        scalar1=b_t[:, hc:hc + 1])
        # V rows with appended ones column: [P(t), kblk, head, Dh+1]
        VR = acts.tile([P, S // P, HL, Dh + 1], BF16, tag="VR")
        nc.vector.memset(VR[:, :, :, Dh:Dh + 1], 1.0)
        for tcN in range(S // P):
            ps = psC.tile([P, DL], FP32, tag="psC")
            for cc in range(CC):
                nc.tensor.matmul(
                    ps, xb[:, cc, tcN * P:(tcN + 1) * P], wv_t[:, cc, :],
                    start=(cc == 0), stop=(cc == CC - 1))
            nc.vector.tensor_add(
                out=VR[:, tcN, :, 0:Dh],
                in0=ps.rearrange("p (h d) -> p h d", h=HL),
                in1=bvB.rearrange("p (h d) -> p h d", h=HL))

        # ---- attention ----
        attnT = acts.tile([P, 2, S], BF16, tag="attnT")
        for h in range(HL):
            hp, ho = h // 2, (h % 2) * Dh
            qt_h = QT[ho:ho + Dh, hp, :]
            kt_h = KT[ho:ho + Dh, hp, :]
            for qg in range(S // 512):
                av = psB.tile([Dh + 1, 512], FP32, tag="psB")
                kmax = qg * 4 + 3
                qsl = slice(qg * 512, (qg + 1) * 512)
                for kb0 in range(0, kmax + 1, 2):
                    npair = min(2, kmax + 1 - kb0)
                    sc = psA.tile([P, 1024], FP32, tag="psA")
                    for j in range(npair):
                        nc.tensor.matmul(sc[:, j * 512:(j + 1) * 512],
                                         kt_h[:, (kb0 + j) * P:(kb0 + j + 1) * P],
                                         qt_h[:, qsl], start=True, stop=True)
                    ex = expp.tile([P, 1024], BF16, tag="ex")
                    nc.scalar.activation(out=ex[:, :npair * 512], in_=sc[:, :npair * 512],
                                         func=Exp, scale=1.0 / 8.0)
                    for j in range(npair):
                        kb = kb0 + j
                        exj = ex[:, j * 512:(j + 1) * 512]
                        dj = kb - qg * 4
                        if 0 <= dj <= 3:  # diagonal block: apply causal mask
                            nc.vector.tensor_mul(out=exj[:, dj * P:(dj + 1) * P],
                                                 in0=exj[:, dj * P:(dj + 1) * P],
                                                 in1=trimask)
                        off = max(0, dj) * P  # columns q >= kb are the only valid ones
                        nc.tensor.matmul(av[:, off:], VR[:, kb, h, :], exj[:, off:],
                                         start=(kb == 0), stop=(kb == kmax))
                nc.vector.tensor_copy(
                    out=attnT[ho:ho + Dh, hp, qg * 512:(qg + 1) * 512],
                    in_=av[0:Dh, :])
                # sum-of-exp lives on partition Dh; evict/recip there, bounce to DRAM
                ev = small.tile([Dh + 1, 512], FP32, tag="sum_ev")
                nc.vector.tensor_copy(out=ev[Dh:Dh + 1, :], in_=av[Dh:Dh + 1, :])
                nc.vector.reciprocal(out=ev[Dh:Dh + 1, :], in_=ev[Dh:Dh + 1, :])
                nc.sync.dma_start(
                    out=D_["rcb"][l].ap()[h:h + 1, qg * 512:(qg + 1) * 512],
                    in_=ev[Dh:Dh + 1, :])
        # normalize attnT by broadcasting 1/sum over the 64 head partitions
        for h in range(HL):
            hp, ho = h // 2, (h % 2) * Dh
            for qg in range(S // 512):
                rq = bcast.tile([P, 512], FP32, tag="recB")
                nc.sync.dma_start(out=rq, in_=bass.AP(tensor=D_["rcb"][l],
                                                      offset=h * S + qg * 512,
                                                      ap=[[0, P], [1, 512]]))
                sl = slice(qg * 512, (qg + 1) * 512)
                nc.vector.tensor_mul(out=attnT[ho:ho + Dh, hp, sl],
                                     in0=attnT[ho:ho + Dh, hp, sl],
                                     in1=rq[ho:ho + Dh, :])

        # ---- O-projection partial -> DRAM -> split ReduceScatter ----
        # tg order [0,2,1,3]: sub-a of both halves first, so RS-a overlaps the
        # rest of the O-projection.
        for tg in (0, 2, 1, 3):
            s = "a" if tg % 2 == 0 else "b"
            for dc in range(CC):
                ps = psC.tile([P, 512], FP32, tag="psC")
                for hc in range(2):
                    nc.tensor.matmul(ps, wo_t[:, hc, dc * P:(dc + 1) * P],
                                     attnT[:, hc, tg * 512:(tg + 1) * 512],
                                     start=(hc == 0), stop=(hc == 1))
                ob = small.tile([P, 512], FP32, tag="o_evict")
                nc.vector.tensor_copy(out=ob, in_=ps)
                nc.sync.dma_start(
                    out=D_[f"apart{s}"][l].ap()[tg // 2, dc * P:(dc + 1) * P, :],
                    in_=ob)
            if tg in (2, 3):
                nc.gpsimd.collective_compute(
                    kind="ReduceScatter", op=ADD, replica_groups=GROUPS,
                    ins=[D_[f"apart{s}"][l].ap()], outs=[D_[f"aown{s}"][l].ap()])

        # ---- residual 1 + LN-A on own half ----
        s1 = halves.tile([P, CC, TOWN], FP32, tag="s1fo")
        for s, sub in (("a", 0), ("b", 1)):
            nc.sync.dma_start(
                out=s1[:, :, sub * HT:(sub + 1) * HT],
                in_=D_[f"aown{s}"][l].ap().rearrange("(cc p) t -> p cc t", p=P))
        for cc in range(CC):
            nc.vector.tensor_scalar_add(out=s1[:, cc, :], in0=s1[:, cc, :],
                                        scalar1=bo_t[:, cc:cc + 1])
            nc.vector.tensor_add(out=s1[:, cc, :], in0=s1[:, cc, :], in1=x_own[:, cc, :])
        y32 = halves.tile([P, CC, TOWN], FP32, tag="y32")
        yb = halves.tile([P, CC, TOWN], BF16, tag="yb")
        layer_norm(s1, D_["stb"][2 * l], y32, yb)

        # ---- FFN on own half (full F), fused per token-group ----
        fo = halves.tile([P, CC, TOWN], FP32, tag="s1fo")
        for tg in range(TOWN // 512):
            sl = slice(tg * 512, (tg + 1) * 512)
            h1T = acts.tile([P, FC, 512], BF16, tag="xb", name="h1T")
            for fc in range(FC):
                w1c = wstrm.tile([P, CC, P], BF16, tag="w1c")
                nc.sync.dma_start(
                    out=w1c,
                    in_=D_["w1"].ap()[l].rearrange("(cc p) f -> p cc f", p=P)[:, :, fc * P:(fc + 1) * P])
                ps = psC.tile([P, 512], FP32, tag="psC")
                for cc in range(CC):
                    nc.tensor.matmul(ps, w1c[:, cc, :], yb[:, cc, sl],
                                     start=(cc == 0), stop=(cc == CC - 1))
                nc.scalar.activation(out=h1T[:, fc, :], in_=ps, func=Relu,
                                     bias=b1_t[:, fc:fc + 1])
            for dc in range(CC):
                w2c = wstrm.tile([P, FC, P], BF16, tag="w2c")
                nc.sync.dma_start(
                    out=w2c,
                    in_=D_["w2"].ap()[l].rearrange("(fc p) d -> p fc d", p=P)[:, :, dc * P:(dc + 1) * P])
                ps = psC.tile([P, 512], FP32, tag="psC")
                for fc in range(FC):
                    nc.tensor.matmul(ps, w2c[:, fc, :], h1T[:, fc, :],
                                     start=(fc == 0), stop=(fc == FC - 1))
                nc.scalar.activation(out=fo[:, dc, sl], in_=ps, func=Relu,
                                     bias=b2_t[:, dc:dc + 1])

        # ---- residual 2 + LN-B -> new x_own (+ bf16 copy for AllGather) ----
        nc.vector.tensor_add(out=fo, in0=fo, in1=y32)
        x_own = stream.tile([P, CC, TOWN], FP32, tag="x_own")
        xhb = halves.tile([P, CC, TOWN], BF16, tag="xhb", name="xhb") if l < L - 1 else None
        layer_norm(fo, D_["stb"][2 * l + 1], x_own, xhb)

        if l < L - 1:
            for s, sub in (("a", 0), ("b", 1)):
                nc.sync.dma_start(
                    out=D_[f"xh{s}"][l + 1].ap().rearrange("(cc p) t -> p cc t", p=P),
                    in_=xhb[:, :, sub * HT:(sub + 1) * HT])
                nc.gpsimd.collective_compute(
                    kind="AllGather", op=mybir.AluOpType.bypass, replica_groups=GROUPS,
                    ins=[D_[f"xh{s}"][l + 1].ap()], outs=[D_[f"xf{s}"][l + 1].ap()])

    # ---- output: transpose x_own back to rows [TOWN, D] ----
    for tb in range(TOWN // P):
        rows = acts.tile([P, D], FP32, tag="rows")
        for cc in range(CC):
            pt = psC.tile([P, P], FP32, tag="psC")
            nc.tensor.transpose(pt, x_own[:, cc, tb * P:(tb + 1) * P], ident)
            nc.vector.tensor_copy(out=rows[:, cc * P:(cc + 1) * P], in_=pt)
        nc.sync.dma_start(out=D_["out"].ap()[tb * P:(tb + 1) * P, :], in_=rows)

    ctx.close()


def _get_program():
    if "nc" not in _CACHED:
        _CACHED["nc"] = _build_program()
    return _CACHED["nc"]


def prep_in_maps(inputs):
    def f32(x):
        return np.ascontiguousarray(np.asarray(x, dtype=np.float32))

    def bf(x):
        return np.ascontiguousarray(np.asarray(x, dtype=np.float32).astype(ml_dtypes.bfloat16))

    source = np.asarray(inputs["source"]).astype(np.int32)
    emb = f32(inputs["emb"])
    ln_g, ln_b = f32(inputs["ln_g"]), f32(inputs["ln_b"])
    w1a, b1a = bf(inputs["w1"]), f32(inputs["b1"])
    w2a, b2a = bf(inputs["w2"]), f32(inputs["b2"])
    wqa, wka, wva = np.asarray(inputs["wq"]), np.asarray(inputs["wk"]), np.asarray(inputs["wv"])
    bqa, bka, bva = np.asarray(inputs["bq"]), np.asarray(inputs["bk"]), np.asarray(inputs["bv"])
    woa, boa = np.asarray(inputs["wo"]), f32(inputs["bo"])

    in_maps = []
    for core in range(8):
        b, half = core // 2, core % 2
        hsl = slice(half * DL, (half + 1) * DL)
        in_maps.append({
            "src": np.ascontiguousarray(source[b, half * TOWN:(half + 1) * TOWN]),
            "emb": emb,
            "wq": bf(wqa[:, :, hsl]), "wk": bf(wka[:, :, hsl]), "wv": bf(wva[:, :, hsl]),
            "bq": f32(bqa[:, hsl]), "bk": f32(bka[:, hsl]), "bv": f32(bva[:, hsl]),
            "wo": bf(woa[:, hsl, :]), "bo": boa,
            "w1": w1a, "b1": b1a, "w2": w2a, "b2": b2a,
            "ln_g": ln_g, "ln_b": ln_b,
        })
    return in_maps


def kernel(**inputs):
    nc = _get_program()
    in_maps = prep_in_maps(inputs)
    trace = bool(int(os.environ.get("BASS_ENC_TRACE", "0")))
    res = bass_utils.run_bass_kernel_spmd(nc, in_maps, core_ids=list(range(8)),
                                          trace=trace)
    _CACHED["last_results"] = res

    outp = np.empty((B, S, D), np.float32)
    for core in range(8):
        b, half = core // 2, core % 2
        outp[b, half * TOWN:(half + 1) * TOWN, :] = res.results[core]["out"]
    return outp


# revision 28
# speedup vs baseline: 4311.8617x; 4311.8617x over previous
"""Trainium2 Bass kernel for nn_Encoder (6-layer causal transformer encoder).

Sharding: 8 cores = 4 batch elements x 2-core tensor-parallel pairs.
Within a pair: attention is head-split (4 of 8 heads per core), FFN/LN/residual
are token-split (1024 of 2048 tokens per core).  Rank asymmetry is expressed
purely through ReduceScatter / AllGather rank order, so the SPMD program is
identical on every core.

Layout: activations are kept feature-major ("xT": [D on partitions, T free]),
which lets every matmul consume its operands without PE transposes and keeps
softmax score blocks in the AV-friendly [k, q] orientation.
"""

import os
import sys

sys.path.insert(0, "/opt/trn_rl_repo")

import numpy as np
import ml_dtypes

import concourse.bass as bass
import concourse.mybir as mybir
import concourse.tile as tile
from concourse import bacc, bass_utils
from concourse.masks import make_identity, make_upper_triangular

# Problem constants (hardcoded per harness contract).
B, S, V, D, F, L = 4, 2048, 32000, 512, 2048, 6
H, Dh = 8, 64
HL = H // 2            # local heads per core (4)
DL = HL * Dh           # 256 local head-dims
TOWN = S // 2          # 1024 tokens owned per core
P = 128
CC = D // P            # 4 c-chunks
FC = F // P            # 16 f-chunks
LN_EPS = 1e-5

FP32 = mybir.dt.float32
BF16 = mybir.dt.bfloat16
I32 = mybir.dt.int32

GROUPS = [[0, 1], [2, 3], [4, 5], [6, 7]]

_CACHED = {}


def _build_program(no_cc=False):
    nc = bacc.Bacc("TRN2", target_bir_lowering=False, debug=False, num_devices=8)
    if no_cc:
        # benchmarking variant: collectives replaced by a local DRAM copy
        # (wrong results; identical compute/DMA structure)
        real_cc = nc.gpsimd.collective_compute

        def fake_cc(kind, op, replica_groups, ins, outs, **kw):
            src = ins[0]
            dst = outs[0]
            n = min(src.size(), dst.size())
            nc.sync.dma_start(
                out=bass.AP(tensor=dst.tensor, offset=0, ap=[[1, n]]),
                in_=bass.AP(tensor=src.tensor, offset=0, ap=[[1, n]]))

        nc.gpsimd.collective_compute = fake_cc

    D_ = {}
    D_["src"] = nc.dram_tensor("src", [TOWN], I32, kind="ExternalInput")
    D_["emb"] = nc.dram_tensor("emb", [V, D], FP32, kind="ExternalInput")
    D_["wq"] = nc.dram_tensor("wq", [L, D, DL], BF16, kind="ExternalInput")
    D_["wk"] = nc.dram_tensor("wk", [L, D, DL], BF16, kind="ExternalInput")
    D_["wv"] = nc.dram_tensor("wv", [L, D, DL], BF16, kind="ExternalInput")
    D_["wo"] = nc.dram_tensor("wo", [L, DL, D], BF16, kind="ExternalInput")
    D_["bq"] = nc.dram_tensor("bq", [L, DL], FP32, kind="ExternalInput")
    D_["bk"] = nc.dram_tensor("bk", [L, DL], FP32, kind="ExternalInput")
    D_["bv"] = nc.dram_tensor("bv", [L, DL], FP32, kind="ExternalInput")
    D_["bo"] = nc.dram_tensor("bo", [L, D], FP32, kind="ExternalInput")
    D_["w1"] = nc.dram_tensor("w1", [L, D, F], BF16, kind="ExternalInput")
    D_["b1"] = nc.dram_tensor("b1", [L, F], FP32, kind="ExternalInput")
    D_["w2"] = nc.dram_tensor("w2", [L, F, D], BF16, kind="ExternalInput")
    D_["b2"] = nc.dram_tensor("b2", [L, D], FP32, kind="ExternalInput")
    D_["ln_g"] = nc.dram_tensor("ln_g", [D], FP32, kind="ExternalInput")
    D_["ln_b"] = nc.dram_tensor("ln_b", [D], FP32, kind="ExternalInput")
    D_["out"] = nc.dram_tensor("out", [TOWN, D], FP32, kind="ExternalOutput")

    # DRAM scratch (one set per layer so layers can overlap freely).
    D_["xh"] = [nc.dram_tensor(f"xh{l}", [D, TOWN], BF16, kind="Internal")
                for l in range(L)]
    D_["xf"] = [nc.dram_tensor(f"xf{l}", [2, D, TOWN], BF16, kind="Internal")
                for l in range(L)]
    D_["apart"] = [nc.dram_tensor(f"apart{l}", [2, D, TOWN], BF16, kind="Internal")
                   for l in range(L)]
    D_["aown"] = [nc.dram_tensor(f"aown{l}", [D, TOWN], BF16, kind="Internal")
                  for l in range(L)]
    D_["stb"] = [nc.dram_tensor(f"stb{l}", [2, TOWN], FP32, kind="Internal")
                 for l in range(2 * L)]
    D_["rcb"] = [nc.dram_tensor(f"rcb{l}", [HL, S], FP32, kind="Internal")
                 for l in range(L)]

    with tile.TileContext(nc) as tc:
        _emit(nc, tc, D_)

    nc.compile()
    return nc


def _emit(nc, tc, D_):
    from contextlib import ExitStack

    ctx = ExitStack()
    Exp = mybir.ActivationFunctionType.Exp
    Relu = mybir.ActivationFunctionType.Relu
    Sqrt = mybir.ActivationFunctionType.Sqrt
    ADD = mybir.AluOpType.add
    MULT = mybir.AluOpType.mult

    consts = ctx.enter_context(tc.tile_pool(name="consts", bufs=1))
    wpool = ctx.enter_context(tc.tile_pool(name="weights", bufs=1))
    wstrm = ctx.enter_context(tc.tile_pool(name="wstrm", bufs=3))
    stream = ctx.enter_context(tc.tile_pool(name="stream", bufs=1))
    acts = ctx.enter_context(tc.tile_pool(name="acts", bufs=1))
    halves = ctx.enter_context(tc.tile_pool(name="halves", bufs=1))
    small = ctx.enter_context(tc.tile_pool(name="small", bufs=2))
    expp = ctx.enter_context(tc.tile_pool(name="exp", bufs=4))
    bcast = ctx.enter_context(tc.tile_pool(name="bcast", bufs=2))
    psA = ctx.enter_context(tc.tile_pool(name="psA", bufs=2, space="PSUM"))
    psB = ctx.enter_context(tc.tile_pool(name="psB", bufs=1, space="PSUM"))
    psC = ctx.enter_context(tc.tile_pool(name="psC", bufs=3, space="PSUM"))

    # ---- constants ----
    ident = consts.tile([P, P], FP32)
    make_identity(nc, ident)
    trimask = consts.tile([P, P], BF16)  # 1 where k<=q
    make_upper_triangular(nc, trimask, val=1.0, diag=True)
    gT = consts.tile([P, CC], FP32)
    nc.sync.dma_start(out=gT, in_=D_["ln_g"].ap().rearrange("(cc p) -> p cc", p=P))
    bT = consts.tile([P, CC], FP32)
    nc.sync.dma_start(out=bT, in_=D_["ln_b"].ap().rearrange("(cc p) -> p cc", p=P))
    ones1 = consts.tile([P, 1], BF16)
    nc.vector.memset(ones1, 1.0)
    epst = consts.tile([1, 1], FP32)
    nc.vector.memset(epst, LN_EPS)
    idx = consts.tile([P, TOWN // P], I32)
    nc.sync.dma_start(out=idx, in_=D_["src"].ap().rearrange("(tc p) -> p tc", p=P))

    # ---- layer norm on own half, feature-major ----
    # s [P, CC, TOWN] fp32 -> out32 (fp32), outb (bf16 or None)
    def layer_norm(s, stb, out32, outb):
        for tg in range(TOWN // 512):
            sl = slice(tg * 512, (tg + 1) * 512)
            ps_m = psC.tile([1, 512], FP32, tag="psC", name="ps_m")
            ps_q = psC.tile([1, 512], FP32, tag="psC", name="ps_q")
            for cc in range(CC):
                chunk = small.tile([P, 512], BF16, tag="ln_chunk")
                nc.vector.tensor_copy(out=chunk, in_=s[:, cc, sl])
                sqc = small.tile([P, 512], BF16, tag="ln_sqc")
                nc.vector.tensor_mul(out=sqc, in0=chunk, in1=chunk)
                nc.tensor.matmul(ps_m, ones1, chunk, start=(cc == 0), stop=(cc == CC - 1))
                nc.tensor.matmul(ps_q, ones1, sqc, start=(cc == 0), stop=(cc == CC - 1))
            mean = small.tile([1, 512], FP32, tag="ln_mean", bufs=2)
            nc.scalar.mul(out=mean, in_=ps_m, mul=1.0 / D)
            msq = small.tile([1, 512], FP32, tag="ln_msq", bufs=1)
            nc.scalar.mul(out=msq, in_=ps_q, mul=1.0 / D)  # E[x^2]
            m2 = small.tile([1, 512], FP32, tag="ln_m2", bufs=1)
            nc.vector.tensor_mul(out=m2, in0=mean, in1=mean)
            nc.vector.tensor_sub(out=msq, in0=msq, in1=m2)
            rstd = small.tile([1, 512], FP32, tag="ln_rstd", bufs=2)
            nc.scalar.activation(out=rstd, in_=msq, func=Sqrt, bias=epst, scale=1.0)
            nc.vector.reciprocal(out=rstd, in_=rstd)
            nc.sync.dma_start(out=stb.ap()[0:1, sl], in_=mean)
            nc.sync.dma_start(out=stb.ap()[1:2, sl], in_=rstd)
        for tg in range(TOWN // 512):
            sl = slice(tg * 512, (tg + 1) * 512)
            mB = bcast.tile([P, 512], FP32, tag="mB")
            nc.sync.dma_start(out=mB, in_=bass.AP(tensor=stb, offset=tg * 512,
                                                  ap=[[0, P], [1, 512]]))
            rB = bcast.tile([P, 512], FP32, tag="rB")
            nc.sync.dma_start(out=rB, in_=bass.AP(tensor=stb, offset=TOWN + tg * 512,
                                                  ap=[[0, P], [1, 512]]))
            for cc in range(CC):
                o = out32[:, cc, sl]
                nc.vector.tensor_sub(out=o, in0=s[:, cc, sl], in1=mB)
                nc.vector.tensor_mul(out=o, in0=o, in1=rB)
                nc.vector.tensor_scalar(out=o, in0=o,
                                        scalar1=gT[:, cc:cc + 1], scalar2=bT[:, cc:cc + 1],
                                        op0=MULT, op1=ADD)
                if outb is not None:
                    nc.vector.tensor_copy(out=outb[:, cc, sl], in_=o)

    # ---- embedding gather for own tokens -> x_own [P, CC, TOWN] fp32 ----
    x_own = stream.tile([P, CC, TOWN], FP32, tag="x_own")
    xhb = halves.tile([P, CC, TOWN], BF16, tag="xhb")
    for tcN in range(TOWN // P):
        rows = acts.tile([P, D], FP32, tag="rows")
        nc.gpsimd.indirect_dma_start(
            out=rows, out_offset=None, in_=D_["emb"].ap(),
            in_offset=bass.IndirectOffsetOnAxis(ap=idx[:, tcN:tcN + 1], axis=0))
        for cc in range(CC):
            pt = psC.tile([P, P], FP32, tag="psC")
            nc.tensor.transpose(pt, rows[:, cc * P:(cc + 1) * P], ident)
            nc.vector.tensor_copy(out=x_own[:, cc, tcN * P:(tcN + 1) * P], in_=pt)
            nc.vector.tensor_copy(out=xhb[:, cc, tcN * P:(tcN + 1) * P], in_=pt)
    nc.sync.dma_start(out=D_["xh"][0].ap().rearrange("(cc p) t -> p cc t", p=P), in_=xhb)
    nc.gpsimd.collective_compute(
        kind="AllGather", op=mybir.AluOpType.bypass, replica_groups=GROUPS,
        ins=[D_["xh"][0].ap()], outs=[D_["xf"][0].ap()])

    for l in range(L):
        # ---- per-layer weights (small ones resident; w1/w2 streamed) ----
        wq_t = wpool.tile([P, CC, DL], BF16, tag="wq")
        nc.sync.dma_start(out=wq_t, in_=D_["wq"].ap()[l].rearrange("(cc p) d -> p cc d", p=P))
        wk_t = wpool.tile([P, CC, DL], BF16, tag="wk")
        nc.sync.dma_start(out=wk_t, in_=D_["wk"].ap()[l].rearrange("(cc p) d -> p cc d", p=P))
        wv_t = wpool.tile([P, CC, DL], BF16, tag="wv")
        nc.sync.dma_start(out=wv_t, in_=D_["wv"].ap()[l].rearrange("(cc p) d -> p cc d", p=P))
        wo_t = wpool.tile([P, 2, D], BF16, tag="wo")
        nc.sync.dma_start(out=wo_t, in_=D_["wo"].ap()[l].rearrange("(hc p) d -> p hc d", p=P))
        bq_t = wpool.tile([P, 2], FP32, tag="bq")
        nc.sync.dma_start(out=bq_t, in_=D_["bq"].ap()[l].rearrange("(hc p) -> p hc", p=P))
        bk_t = wpool.tile([P, 2], FP32, tag="bk")
        nc.sync.dma_start(out=bk_t, in_=D_["bk"].ap()[l].rearrange("(hc p) -> p hc", p=P))
        bvB = wpool.tile([P, DL], FP32, tag="bvB")
        nc.sync.dma_start(out=bvB, in_=bass.AP(tensor=D_["bv"], offset=l * DL,
                                               ap=[[0, P], [1, DL]]))
        bo_t = wpool.tile([P, CC], FP32, tag="bo")
        nc.sync.dma_start(out=bo_t, in_=D_["bo"].ap()[l].rearrange("(cc p) -> p cc", p=P))
        b1_t = wpool.tile([P, FC], FP32, tag="b1")
        nc.sync.dma_start(out=b1_t, in_=D_["b1"].ap()[l].rearrange("(fc p) -> p fc", p=P))
        b2_t = wpool.tile([P, CC], FP32, tag="b2")
        nc.sync.dma_start(out=b2_t, in_=D_["b2"].ap()[l].rearrange("(cc p) -> p cc", p=P))

        # ---- gathered x (full sequence, bf16) ----
        xb = acts.tile([P, CC, S], BF16, tag="xb")
        for half in range(2):
            nc.sync.dma_start(
                out=xb[:, :, half * TOWN:(half + 1) * TOWN],
                in_=D_["xf"][l].ap()[half].rearrange("(cc p) t -> p cc t", p=P))

        # ---- QKV projections ----
        QT = acts.tile([P, 2, S], BF16, tag="QT")
        KT = acts.tile([P, 2, S], BF16, tag="KT")
        for dst, w_t, b_t in ((QT, wq_t, bq_t), (KT, wk_t, bk_t)):
            for hc in range(2):
                for tg in range(S // 512):
                    ps = psC.tile([P, 512], FP32, tag="psC")
                    for cc in range(CC):
                        nc.tensor.matmul(
                            ps, w_t[:, cc, hc * P:(hc + 1) * P],
                            xb[:, cc, tg * 512:(tg + 1) * 512],
                            start=(cc == 0), stop=(cc == CC - 1))
                    nc.vector.tensor_scalar_add(
                        out=dst[:, hc, tg * 512:(tg + 1) * 512], in0=ps,
                        scalar1=b_t[:, hc:hc + 1])
        # V rows with appended ones column: [P(t), kblk, head, Dh+1]
        VR = acts.tile([P, S // P, HL, Dh + 1], BF16, tag="VR")
        nc.vector.memset(VR[:, :, :, Dh:Dh + 1], 1.0)
        for tcN in range(S // P):
            ps = psC.tile([P, DL], FP32, tag="psC")
            for cc in range(CC):
                nc.tensor.matmul(
                    ps, xb[:, cc, tcN * P:(tcN + 1) * P], wv_t[:, cc, :],
                    start=(cc == 0), stop=(cc == CC - 1))
            nc.vector.tensor_add(
                out=VR[:, tcN, :, 0:Dh],
                in0=ps.rearrange("p (h d) -> p h d", h=HL),
                in1=bvB.rearrange("p (h d) -> p h d", h=HL))

        # ---- attention ----
        attnT = acts.tile([P, 2, S], BF16, tag="attnT")
        for h in range(HL):
            hp, ho = h // 2, (h % 2) * Dh
            qt_h = QT[ho:ho + Dh, hp, :]
            kt_h = KT[ho:ho + Dh, hp, :]
            for qg in range(S // 512):
                av = psB.tile([Dh + 1, 512], FP32, tag="psB")
                kmax = qg * 4 + 3
                qsl = slice(qg * 512, (qg + 1) * 512)
                for kb0 in range(0, kmax + 1, 2):
                    npair = min(2, kmax + 1 - kb0)
                    sc = psA.tile([P, 1024], FP32, tag="psA")
                    for j in range(npair):
                        nc.tensor.matmul(sc[:, j * 512:(j + 1) * 512],
                                         kt_h[:, (kb0 + j) * P:(kb0 + j + 1) * P],
                                         qt_h[:, qsl], start=True, stop=True)
                    ex = expp.tile([P, 1024], BF16, tag="ex")
                    nc.scalar.activation(out=ex[:, :npair * 512], in_=sc[:, :npair * 512],
                                         func=Exp, scale=1.0 / 8.0)
                    for j in range(npair):
                        kb = kb0 + j
                        exj = ex[:, j * 512:(j + 1) * 512]
                        dj = kb - qg * 4
                        if 0 <= dj <= 3:  # diagonal block: apply causal mask
                            nc.vector.tensor_mul(out=exj[:, dj * P:(dj + 1) * P],
                                                 in0=exj[:, dj * P:(dj + 1) * P],
                                                 in1=trimask)
                        off = max(0, dj) * P  # columns q >= kb are the only valid ones
                        nc.tensor.matmul(av[:, off:], VR[:, kb, h, :], exj[:, off:],
                                         start=(kb == 0), stop=(kb == kmax))
                nc.vector.tensor_copy(
                    out=attnT[ho:ho + Dh, hp, qg * 512:(qg + 1) * 512],
                    in_=av[0:Dh, :])
                # sum-of-exp lives on partition Dh; evict/recip there, bounce to DRAM
                ev = small.tile([Dh + 1, 512], FP32, tag="sum_ev")
                nc.vector.tensor_copy(out=ev[Dh:Dh + 1, :], in_=av[Dh:Dh + 1, :])
                nc.vector.reciprocal(out=ev[Dh:Dh + 1, :], in_=ev[Dh:Dh + 1, :])
                nc.sync.dma_start(
                    out=D_["rcb"][l].ap()[h:h + 1, qg * 512:(qg + 1) * 512],
                    in_=ev[Dh:Dh + 1, :])
        # normalize attnT by broadcasting 1/sum over the 64 head partitions
        for h in range(HL):
            hp, ho = h // 2, (h % 2) * Dh
            for qg in range(S // 512):
                rq = bcast.tile([P, 512], FP32, tag="recB")
                nc.sync.dma_start(out=rq, in_=bass.AP(tensor=D_["rcb"][l],
                                                      offset=h * S + qg * 512,
                                                      ap=[[0, P], [1, 512]]))
                sl = slice(qg * 512, (qg + 1) * 512)
                nc.vector.tensor_mul(out=attnT[ho:ho + Dh, hp, sl],
                                     in0=attnT[ho:ho + Dh, hp, sl],
                                     in1=rq[ho:ho + Dh, :])

        # ---- O-projection partial (bf16) -> DRAM -> ReduceScatter ----
        for tg in range(S // 512):
            for dc in range(CC):
                ps = psC.tile([P, 512], FP32, tag="psC")
                for hc in range(2):
                    nc.tensor.matmul(ps, wo_t[:, hc, dc * P:(dc + 1) * P],
                                     attnT[:, hc, tg * 512:(tg + 1) * 512],
                                     start=(hc == 0), stop=(hc == 1))
                ob = small.tile([P, 512], BF16, tag="o_evict")
                nc.vector.tensor_copy(out=ob, in_=ps)
                nc.sync.dma_start(
                    out=D_["apart"][l].ap()[tg // 2, dc * P:(dc + 1) * P,
                                            (tg % 2) * 512:((tg % 2) + 1) * 512],
                    in_=ob)
        nc.gpsimd.collective_compute(
            kind="ReduceScatter", op=ADD, replica_groups=GROUPS,
            ins=[D_["apart"][l].ap()], outs=[D_["aown"][l].ap()])

        # ---- residual 1 + LN-A on own half ----
        ar = halves.tile([P, CC, TOWN], BF16, tag="xhb", name="ar")
        nc.sync.dma_start(out=ar, in_=D_["aown"][l].ap().rearrange("(cc p) t -> p cc t", p=P))
        s1 = halves.tile([P, CC, TOWN], FP32, tag="s1fo")
        for cc in range(CC):
            nc.vector.tensor_scalar_add(out=s1[:, cc, :], in0=ar[:, cc, :],
                                        scalar1=bo_t[:, cc:cc + 1])
            nc.vector.tensor_add(out=s1[:, cc, :], in0=s1[:, cc, :], in1=x_own[:, cc, :])
        y32 = halves.tile([P, CC, TOWN], FP32, tag="y32")
        yb = halves.tile([P, CC, TOWN], BF16, tag="yb")
        layer_norm(s1, D_["stb"][2 * l], y32, yb)

        # ---- FFN on own half (full F), fused per token-group ----
        fo = halves.tile([P, CC, TOWN], FP32, tag="s1fo")
        for tg in range(TOWN // 512):
            sl = slice(tg * 512, (tg + 1) * 512)
            h1T = acts.tile([P, FC, 512], BF16, tag="xb", name="h1T")
            for fc in range(FC):
                w1c = wstrm.tile([P, CC, P], BF16, tag="w1c")
                nc.sync.dma_start(
                    out=w1c,
                    in_=D_["w1"].ap()[l].rearrange("(cc p) f -> p cc f", p=P)[:, :, fc * P:(fc + 1) * P])
                ps = psC.tile([P, 512], FP32, tag="psC")
                for cc in range(CC):
                    nc.tensor.matmul(ps, w1c[:, cc, :], yb[:, cc, sl],
                                     start=(cc == 0), stop=(cc == CC - 1))
                nc.scalar.activation(out=h1T[:, fc, :], in_=ps, func=Relu,
                                     bias=b1_t[:, fc:fc + 1])
            for dc in range(CC):
                w2c = wstrm.tile([P, FC, P], BF16, tag="w2c")
                nc.sync.dma_start(
                    out=w2c,
                    in_=D_["w2"].ap()[l].rearrange("(fc p) d -> p fc d", p=P)[:, :, dc * P:(dc + 1) * P])
                ps = psC.tile([P, 512], FP32, tag="psC")
                for fc in range(FC):
                    nc.tensor.matmul(ps, w2c[:, fc, :], h1T[:, fc, :],
                                     start=(fc == 0), stop=(fc == FC - 1))
                nc.scalar.activation(out=fo[:, dc, sl], in_=ps, func=Relu,
                                     bias=b2_t[:, dc:dc + 1])

        # ---- residual 2 + LN-B -> new x_own (+ bf16 copy for AllGather) ----
        nc.vector.tensor_add(out=fo, in0=fo, in1=y32)
        x_own = stream.tile([P, CC, TOWN], FP32, tag="x_own")
        xhb = halves.tile([P, CC, TOWN], BF16, tag="xhb", name="xhb") if l < L - 1 else None
        layer_norm(fo, D_["stb"][2 * l + 1], x_own, xhb)

        if l < L - 1:
            nc.sync.dma_start(out=D_["xh"][l + 1].ap().rearrange("(cc p) t -> p cc t", p=P),
                              in_=xhb)
            nc.gpsimd.collective_compute(
                kind="AllGather", op=mybir.AluOpType.bypass, replica_groups=GROUPS,
                ins=[D_["xh"][l + 1].ap()], outs=[D_["xf"][l + 1].ap()])

    # ---- output: transpose x_own back to rows [TOWN, D] ----
    for tb in range(TOWN // P):
        rows = acts.tile([P, D], FP32, tag="rows")
        for cc in range(CC):
            pt = psC.tile([P, P], FP32, tag="psC")
            nc.tensor.transpose(pt, x_own[:, cc, tb * P:(tb + 1) * P], ident)
            nc.vector.tensor_copy(out=rows[:, cc * P:(cc + 1) * P], in_=pt)
        nc.sync.dma_start(out=D_["out"].ap()[tb * P:(tb + 1) * P, :], in_=rows)

    ctx.close()


def _get_program():
    no_cc = bool(int(os.environ.get("BASS_ENC_NOCC", "0")))
    key = ("nc", no_cc)
    if key not in _CACHED:
        _CACHED[key] = _build_program(no_cc)
    return _CACHED[key]


def prep_in_maps(inputs):
    def f32(x):
        return np.ascontiguousarray(np.asarray(x, dtype=np.float32))

    def bf(x):
        return np.ascontiguousarray(np.asarray(x, dtype=np.float32).astype(ml_dtypes.bfloat16))

    source = np.asarray(inputs["source"]).astype(np.int32)
    emb = f32(inputs["emb"])
    ln_g, ln_b = f32(inputs["ln_g"]), f32(inputs["ln_b"])
    w1a, b1a = bf(inputs["w1"]), f32(inputs["b1"])
    w2a, b2a = bf(inputs["w2"]), f32(inputs["b2"])
    wqa, wka, wva = np.asarray(inputs["wq"]), np.asarray(inputs["wk"]), np.asarray(inputs["wv"])
    bqa, bka, bva = np.asarray(inputs["bq"]), np.asarray(inputs["bk"]), np.asarray(inputs["bv"])
    woa, boa = np.asarray(inputs["wo"]), f32(inputs["bo"])

    in_maps = []
    for core in range(8):
        b, half = core // 2, core % 2
        hsl = slice(half * DL, (half + 1) * DL)
        in_maps.append({
            "src": np.ascontiguousarray(source[b, half * TOWN:(half + 1) * TOWN]),
            "emb": emb,
            "wq": bf(wqa[:, :, hsl]), "wk": bf(wka[:, :, hsl]), "wv": bf(wva[:, :, hsl]),
            "bq": f32(bqa[:, hsl]), "bk": f32(bka[:, hsl]), "bv": f32(bva[:, hsl]),
            "wo": bf(woa[:, hsl, :]), "bo": boa,
            "w1": w1a, "b1": b1a, "w2": w2a, "b2": b2a,
            "ln_g": ln_g, "ln_b": ln_b,
        })
    return in_maps


def kernel(**inputs):
    nc = _get_program()
    in_maps = prep_in_maps(inputs)
    trace = bool(int(os.environ.get("BASS_ENC_TRACE", "0")))
    res = bass_utils.run_bass_kernel_spmd(nc, in_maps, core_ids=list(range(8)),
                                          trace=trace)
    _CACHED["last_results"] = res

    outp = np.empty((B, S, D), np.float32)
    for core in range(8):
        b, half = core // 2, core % 2
        outp[b, half * TOWN:(half + 1) * TOWN, :] = res.results[core]["out"]
    return outp
